# revision 50
# baseline (speedup 1.0000x reference)
"""BitNet-style attention block (ternary-quantized QKV/proj) on 8 Trainium2 cores.

Data-parallel over batch (16 -> 2 per core, no collectives). Feature-major
("transposed") on-chip layout:
  - x staged host-side as x.T per core: [C, T], T = 2048 tokens/core.
  - Weights ternarized HOST-side (t in {-1,0,1}, exact in bf16); the f32
    per-tensor scale s is applied at PSUM evacuation.
  - QKV produces qkv.T; Q/K resident in SBUF, V in natural [tok, d] layout
    augmented with a ones column (row-sum l falls out of the A@V matmul).
  - attention per (batch, head-pair): K-blocks stream flash-style; the two
    heads' QK matmuls share the PE via disjoint row groups; exp runs as ONE
    ACTIVATE over both heads' logits (FD=1024) to amortize ACT overhead.
  - softmax normalization: l rows go SBUF->DRAM, are re-read spread across
    128 partitions, one cheap reciprocal, then broadcast back per head
    (the [1,512]-shaped DVE reciprocal of the previous version was ~4us each).
  - QKV/V/proj matmul chains are hand-interleaved into the ACT-bound
    attention phase as PE gap fillers (PSUM: 4 banks QK-pair ping-pong,
    3 banks AV accumulators, 1 bank filler chains).
"""

import os
import sys

import ml_dtypes
import numpy as np

for _p in ("/opt/trn_rl_repo", "/root/.axon_site/_ro/trn_rl_repo"):
    if os.path.isdir(_p) and _p not in sys.path:
        sys.path.insert(0, _p)

import bass_rust
import concourse.bass as bass
import concourse.mybir as mybir
import concourse.tile as tile
from concourse import bacc
from concourse.bass_utils import run_bass_kernel_spmd

B, N, C, H = 16, 1024, 768, 12
HD = C // H                    # 64
SCALE = float(HD ** -0.5)      # 0.125
EPS = 1e-5
NCORES = 8
BPC = B // NCORES              # 2 batches per core
T = BPC * N                    # 2048 tokens per core
P = 128
CB = C // P                    # 6 c-blocks
MQK = (2 * C) // P             # 12 m-blocks covering Q and K rows of qkv
TB = T // P                    # 16 token blocks
F32 = mybir.dt.float32
BF16 = mybir.dt.bfloat16
FP8 = mybir.dt.float8e4
DR = mybir.MatmulPerfMode.DoubleRow
AF = mybir.ActivationFunctionType
ALU = mybir.AluOpType

_CACHED_NC = None


def _split_drain_waits(nc):
    """The walrus build in this container accepts only one sync-wait per
    instruction; move extra waits onto preceding single-wait NoOps on the
    same engine (in-order queues make this semantics-preserving)."""
    for fn in nc.m.functions:
        for bb in fn.blocks:
            insts = bb.instructions
            i = 0
            while i < len(insts):
                inst = insts[i]
                si = getattr(inst, "sync_info", None)
                if (
                    si is not None
                    and si.on_wait is not None
                    and len(si.on_wait) > 1
                    # DMA waits are enforced at the DGE-queue level, not the
                    # sequencer; hoisting them onto a sequencer NoOp can
                    # deadlock (head-of-line blocking across queues).
                    and not type(inst).__name__.startswith("InstDMA")
                ):
                    waits = list(si.on_wait)
                    for j, w in enumerate(waits[:-1]):
                        nop = mybir.InstNoOp(
                            name=f"{inst.name}-prewait-{j}", ins=[], outs=[]
                        )
                        nop.engine = inst.engine
                        nop.sync_info = mybir.SyncInfo(on_wait=[w], on_update=[])
                        insts.insert(i, nop)
                        i += 1
                    inst.sync_info = mybir.SyncInfo(
                        on_wait=[waits[-1]], on_update=list(si.on_update)
                    )
                i += 1


def _build_nc():
    nc = bacc.Bacc(None)

    xT = nc.dram_tensor("xT", [C, T], BF16, kind="ExternalInput")
    x8T = nc.dram_tensor("x8T", [C, T], FP8, kind="ExternalInput")
    wq8T = nc.dram_tensor("wq8T", [C, 2 * C], FP8, kind="ExternalInput")  # ternary
    wvT = nc.dram_tensor("wvT", [C, C], BF16, kind="ExternalInput")       # ternary
    wpT = nc.dram_tensor("wpT", [C, C], BF16, kind="ExternalInput")       # ternary
    bp = nc.dram_tensor("bp", [C], F32, kind="ExternalInput")
    sq = nc.dram_tensor("sq", [1, 1], F32, kind="ExternalInput")
    sp = nc.dram_tensor("sp", [1, 1], F32, kind="ExternalInput")
    cz = nc.dram_tensor("cz", [2, N], BF16, kind="ExternalInput")  # row1 = 1.0
    yT = nc.dram_tensor("yT", [C, T], F32, kind="ExternalOutput")

    from contextlib import ExitStack

    with tile.TileContext(nc) as tc:
        with ExitStack() as stack:
            ep = stack.enter_context
            constp = ep(tc.tile_pool(name="constp", bufs=1))
            xp = ep(tc.tile_pool(name="xp", bufs=1))
            outp = ep(tc.tile_pool(name="outp", bufs=1))
            wqp = ep(tc.tile_pool(name="wqp", bufs=1))
            wvp = ep(tc.tile_pool(name="wvp", bufs=1))
            wpp = ep(tc.tile_pool(name="wpp", bufs=1))
            qkp = ep(tc.tile_pool(name="qkp", bufs=1))
            vaugp = ep(tc.tile_pool(name="vaugp", bufs=1))
            attnp = ep(tc.tile_pool(name="attnp", bufs=4))
            avsp = ep(tc.tile_pool(name="avsp", bufs=5))
            lspp = ep(tc.tile_pool(name="lspp", bufs=2))
            linvp = ep(tc.tile_pool(name="linvp", bufs=2))
            bcp = ep(tc.tile_pool(name="bcp", bufs=2))
            stagep = ep(tc.tile_pool(name="stagep", bufs=3))
            psp = ep(tc.tile_pool(name="psp", bufs=2, space="PSUM"))
            avp = ep(tc.tile_pool(name="avp", bufs=3, space="PSUM"))
            qkvp = ep(tc.tile_pool(name="qkvp", bufs=1, space="PSUM"))
            dml = ep(tc.tile_pool(name="dml", bufs=2, space="DRAM"))
            dml2 = ep(tc.tile_pool(name="dml2", bufs=2, space="DRAM"))
            sqb = constp.tile([P, 1], F32, tag="sqb")
            spb = constp.tile([P, 1], F32, tag="spb")
            b_sb = constp.tile([P, CB], F32, tag="b_sb")
            ones_col = constp.tile([P, 1], BF16, tag="ones_col")

            # ---- loads ordered for the critical path: the first QKV chains
            # need x8 chunk 0 + wq8; V chains need bf16 x + wv next.
            x_sb = xp.tile([P, CB, T], BF16, tag="x")
            x8_sb = xp.tile([P, CB, T], FP8, tag="x8")
            wq8 = wqp.tile([P, CB, 2 * C], FP8, tag="wq")
            wv_q = wvp.tile([P, CB, C], BF16, tag="wv")
            wp_q = wpp.tile([P, CB, C], BF16, tag="wp")

            def load_x(hg):
                nc.sync.dma_start(
                    x_sb[:, :, hg * 1024 : (hg + 1) * 1024],
                    xT[:, hg * 1024 : (hg + 1) * 1024].rearrange(
                        "(cb p) t -> p cb t", p=P
                    ),
                )

            nc.sync.dma_start(
                x8_sb[:], x8T[:, :].rearrange("(cb p) t -> p cb t", p=P)
            )
            nc.sync.dma_start(
                wq8[:, :, 0:C], wq8T[:, 0:C].rearrange("(cb p) m -> p cb m", p=P)
            )
            nc.sync.dma_start(
                wq8[:, :, C : 2 * C],
                wq8T[:, C : 2 * C].rearrange("(cb p) m -> p cb m", p=P),
            )
            nc.sync.dma_start(sqb[:], sq[:, :].to_broadcast([P, 1]))
            load_x(0)
            nc.sync.dma_start(
                wv_q[:], wvT[:, :].rearrange("(cb p) m -> p cb m", p=P)
            )
            nc.sync.dma_start(ones_col[:], cz[1:2, 0:1].to_broadcast([P, 1]))
            load_x(1)
            nc.sync.dma_start(
                wp_q[:], wpT[:, :].rearrange("(cb p) m -> p cb m", p=P)
            )
            nc.sync.dma_start(spb[:], sp[:, :].to_broadcast([P, 1]))
            nc.sync.dma_start(b_sb[:], bp[:].rearrange("(cb p) -> p cb", p=P))

            # ---- V-augmented tile: ones column ----
            v_aug = vaugp.tile([P, TB, H, HD + 1], BF16, tag="vaug")
            nc.vector.tensor_copy(
                v_aug[:, :, :, HD : HD + 1],
                ones_col[:, None, :].to_broadcast([P, TB, H, 1]),
            )

            qksb = qkp.tile([P, MQK, T], BF16, tag="qksb")
            outT = outp.tile([P, CB, T], BF16, tag="outT")

            # ---------- chain emitters (QKV / V / proj) ----------
            pool_rr = [qkvp, avp, avp, avp, psp, psp]
            rr_idx = [0]

            def next_pool():
                p = pool_rr[rr_idx[0] % len(pool_rr)]
                rr_idx[0] += 1
                return p

            def qkv_chain(mi, qc, pool=None):
                # Q/K rows: fp8 DoubleRow over ci-pairs (ternary weights are
                # exact in fp8; x8 error is softmax-compressed downstream)
                pool = pool or qkvp
                tag = "st" if pool is psp else ("av" if pool is avp else "qkv")
                ps = pool.tile([P, 512], F32, tag=tag, name=f"qkv{mi}_{qc}")
                for cp in range(CB // 2):
                    nc.tensor.matmul(
                        ps[:],
                        wq8[:, 2 * cp : 2 * cp + 2, mi * P : (mi + 1) * P],
                        x8_sb[:, 2 * cp : 2 * cp + 2, qc * 512 : (qc + 1) * 512],
                        start=(cp == 0),
                        stop=(cp == CB // 2 - 1),
                        perf_mode=DR,
                    )
                nc.vector.tensor_scalar_mul(
                    qksb[:, mi, qc * 512 : (qc + 1) * 512], ps[:], sqb[:]
                )

            def v_chain(nch, tb, pool=None):
                pool = pool or qkvp
                tag = "st" if pool is psp else ("av" if pool is avp else "qkv")
                ps = pool.tile([P, 512], F32, tag=tag, name=f"v{nch}_{tb}")
                for ci in range(CB):
                    nc.tensor.matmul(
                        ps[:, :384],
                        x_sb[:, ci, tb * P : (tb + 1) * P],
                        wv_q[:, ci, nch * 384 : (nch + 1) * 384],
                        start=(ci == 0),
                        stop=(ci == CB - 1),
                    )
                nc.vector.tensor_scalar_mul(
                    v_aug[:, tb, nch * 6 : (nch + 1) * 6, 0:HD],
                    ps[:, :384].rearrange("p (h d) -> p h d", d=HD),
                    sqb[:],
                )

            def proj_chain(co, qcg, evac_engine, pool=None):
                pool = pool or qkvp
                tag = "st" if pool is psp else ("av" if pool is avp else "qkv")
                ps = pool.tile([P, 512], F32, tag=tag, name=f"pj{co}_{qcg}")
                for ci in range(CB):
                    nc.tensor.matmul(
                        ps[:],
                        wp_q[:, ci, co * P : (co + 1) * P],
                        outT[:, ci, qcg * 512 : (qcg + 1) * 512],
                        start=(ci == 0),
                        stop=(ci == CB - 1),
                    )
                yst = stagep.tile([P, 512], F32, tag="evac")
                if evac_engine == "act":
                    nc.scalar.activation(
                        yst[:], ps[:], AF.Identity,
                        bias=b_sb[:, co : co + 1], scale=spb[:],
                    )
                else:
                    nc.vector.tensor_scalar(
                        yst[:], ps[:], spb[:], b_sb[:, co : co + 1],
                        ALU.mult, ALU.add,
                    )
                nc.sync.dma_start(
                    yT[co * P : (co + 1) * P, qcg * 512 : (qcg + 1) * 512], yst[:]
                )

            # ---------- filler queue ----------
            fillers = []          # list of closures
            fill_pos = [0]

            def pop_fillers(k):
                n = 0
                while n < k and fill_pos[0] < len(fillers):
                    fillers[fill_pos[0]]()
                    fill_pos[0] += 1
                    n += 1

            def flush_fillers_to(idx):
                while fill_pos[0] < min(idx, len(fillers)):
                    fillers[fill_pos[0]]()
                    fill_pos[0] += 1

            # prologue: only what attention (b0,hp0) strictly needs — the
            # b0 column-halves of Q/K for head-pair 0, plus V for b0 tbs
            qkv_chain(0, 0, next_pool())
            qkv_chain(CB, 0, next_pool())
            qkv_chain(CB, 1, next_pool())
            for tb in range(8):
                v_chain(0, tb, next_pool())
            qkv_chain(0, 1, next_pool())

            # filler queue in consumer order; group (b, hp) only needs the
            # q-halves of its own batch: q0/q1 for b0, q2/q3 for b1
            def add_qk(hp, qcs):
                for mi in (hp, CB + hp):
                    for qc in qcs:
                        fillers.append(lambda mi=mi, qc=qc: qkv_chain(mi, qc))

            need = {}
            for hp in (1, 2):
                add_qk(hp, (0, 1))
                need[(0, hp)] = len(fillers)
            for tb in range(8):
                fillers.append(lambda tb=tb: v_chain(1, tb))
            for hp in (3, 4, 5):
                add_qk(hp, (0, 1))
                need[(0, hp)] = len(fillers)
            add_qk(0, (2, 3))
            for tb in range(8, 16):
                fillers.append(lambda tb=tb: v_chain(0, tb))
            need[(1, 0)] = len(fillers)
            for hp in (1, 2):
                add_qk(hp, (2, 3))
                need[(1, hp)] = len(fillers)
            add_qk(3, (2, 3))
            for tb in range(8, 16):
                fillers.append(lambda tb=tb: v_chain(1, tb))
            need[(1, 3)] = len(fillers)
            for hp in (4, 5):
                add_qk(hp, (2, 3))
                need[(1, hp)] = len(fillers)

            # ---------- attention ----------
            pending_div = [None]
            prev_av = [None]  # last AV matmul, for PE-queue ordering edges

            def attn_group(b, hp, split_l=False):
                ldram = dml.tile([4 * 512], BF16, tag="ld", name=f"ld{b}_{hp}")
                lidram = dml2.tile([4 * 512], BF16, tag="li", name=f"li{b}_{hp}")
                av_sbs = []
                bcs = []
                for qc in range(2):
                    avs = [
                        avp.tile([P, 512], F32, tag="av", name=f"av{b}{hp}{qc}{hh}")
                        for hh in range(2)
                    ]
                    for kb in range(8):
                        st = psp.tile([P, 2, 512], F32, tag="st")
                        for hh in range(2):
                            roff = hh * HD
                            qk_mm = nc.tensor.matmul(
                                st[:, hh, :],
                                qksb[
                                    roff : roff + HD,
                                    CB + hp,
                                    b * N + kb * P : b * N + (kb + 1) * P,
                                ],
                                qksb[
                                    roff : roff + HD,
                                    hp,
                                    b * N + qc * 512 : b * N + qc * 512 + 512,
                                ],
                                start=True,
                                stop=True,
                            )

                        e = attnp.tile([P, 2, 512], BF16, tag="e")
                        nc.scalar.activation(
                            e[:], st[:], AF.Exp, bias=0.0, scale=SCALE
                        )
                        for hh in range(2):
                            av_mm = nc.tensor.matmul(
                                avs[hh][0 : HD + 1, :],
                                v_aug[:, b * 8 + kb, 2 * hp + hh, :],
                                e[:, hh, :],
                                start=(kb == 0),
                                stop=(kb == 7),
                            )
                        prev_av[0] = av_mm
                        if kb in (2, 6):
                            pop_fillers(1)
                    av_sb = avsp.tile([P, 2, 512], BF16, tag="avsb",
                                      name=f"avsb{b}_{hp}_{qc}")
                    for hh in range(2):
                        nc.vector.tensor_copy(
                            av_sb[0 : HD + 1, hh, :], avs[hh][0 : HD + 1, :]
                        )
                    nc.sync.dma_start(
                        ldram[qc * 1024 : (qc + 1) * 1024], av_sb[64:65, :, :]
                    )
                    av_sbs.append(av_sb)
                    if qc == 0 and pending_div[0] is not None:
                        fn, key = pending_div[0]
                        fn()
                        pending_div[0] = None
                        if key == (0, H // 2 - 1):
                            # outT(b0) fully written (in emission order) only
                            # now: proj(b0) fillers are safe to queue
                            for co in range(CB):
                                for qcg in (0, 1):
                                    fillers.append(
                                        lambda co=co, qcg=qcg: proj_chain(
                                            co, qcg, "dve"
                                        )
                                    )
                    if split_l:
                        # last group: per-qc chain to shorten the exposed tail
                        sl = slice(qc * 1024, (qc + 1) * 1024)
                        lsp = lspp.tile([P, 8], BF16, tag="lsp")
                        nc.sync.dma_start(
                            lsp[:], ldram[sl].rearrange("(p f) -> p f", p=P)
                        )
                        linv = linvp.tile([P, 8], BF16, tag="linv")
                        with nc.allow_low_precision(reason="1/l in bf16"):
                            nc.vector.reciprocal(linv[:], lsp[:])
                        nc.sync.dma_start(
                            lidram[sl].rearrange("(p f) -> p f", p=P), linv[:]
                        )
                        bc = bcp.tile([HD, 2, 512], BF16, tag="bc")
                        nc.sync.dma_start(
                            bc[:],
                            lidram[sl]
                            .rearrange("(a c) -> a c", a=2)
                            .partition_broadcast(HD),
                        )
                        bcs.append(bc)
                if not split_l:
                    # l -> spread -> reciprocal -> broadcast, whole group
                    lsp = lspp.tile([P, 16], BF16, tag="lsp")
                    nc.sync.dma_start(lsp[:], ldram[:].rearrange("(p f) -> p f", p=P))
                    linv = linvp.tile([P, 16], BF16, tag="linv")
                    with nc.allow_low_precision(
                        reason="1/l in bf16: 0.4% rel, inside the 2e-2 budget"
                    ):
                        nc.vector.reciprocal(linv[:], lsp[:])
                    nc.sync.dma_start(
                        lidram[:].rearrange("(p f) -> p f", p=P), linv[:]
                    )
                    bc = bcp.tile([HD, 4, 512], BF16, tag="bc")
                    nc.sync.dma_start(
                        bc[:],
                        lidram[:].rearrange("(a c) -> a c", a=4).partition_broadcast(HD),
                    )
                    bcs = [bc, bc]

                def divisions(b=b, hp=hp, av_sbs=av_sbs, bcs=bcs, split=split_l):
                    for qc in range(2):
                        for hh in range(2):
                            col = hh if split else qc * 2 + hh
                            nc.vector.tensor_mul(
                                out=outT[
                                    hh * HD : (hh + 1) * HD,
                                    hp,
                                    b * N + qc * 512 : b * N + qc * 512 + 512,
                                ],
                                in0=av_sbs[qc][0:HD, hh, :],
                                in1=bcs[qc][:, col, :],
                            )

                pending_div[0] = (divisions, (b, hp))

            for b in range(BPC):
                for hp in range(H // 2):
                    flush_fillers_to(need.get((b, hp), 0))
                    attn_group(b, hp, split_l=(b == 1 and hp == 5))

            # tail: last divisions + remaining fillers + proj(b1).
            # Two-pass proj(b1): park qcg=2 chains with ci 0..4 accumulated
            # (ready PE work that fills the last division-chain DMA latency),
            # then finish each with the division-gated ci=5 + evac.
            flush_fillers_to(len(fillers))
            if pending_div[0] is not None:
                pending_div[0][0]()
                pending_div[0] = None
            tail_pools = [qkvp, avp, avp, avp, psp, psp]
            parked = []
            for j, co in enumerate(range(CB)):
                pool = tail_pools[j]
                tag = "st" if pool is psp else ("av" if pool is avp else "qkv")
                ps = pool.tile([P, 512], F32, tag=tag, name=f"pjp{co}")
                for ci in range(CB - 1):
                    nc.tensor.matmul(
                        ps[:],
                        wp_q[:, ci, co * P : (co + 1) * P],
                        outT[:, ci, 2 * 512 : 3 * 512],
                        start=(ci == 0),
                        stop=False,
                    )
                parked.append((ps, co))
            for ps, co in parked:
                nc.tensor.matmul(
                    ps[:],
                    wp_q[:, CB - 1, co * P : (co + 1) * P],
                    outT[:, CB - 1, 2 * 512 : 3 * 512],
                    start=False,
                    stop=True,
                )
                yst = stagep.tile([P, 512], F32, tag="evac")
                nc.scalar.activation(
                    yst[:], ps[:], AF.Identity,
                    bias=b_sb[:, co : co + 1], scale=spb[:],
                )
                nc.sync.dma_start(
                    yT[co * P : (co + 1) * P, 2 * 512 : 3 * 512], yst[:]
                )
            for j, co in enumerate(range(CB)):
                proj_chain(co, 3, "act", pool=tail_pools[j])

    nc.finalize()
    return nc


def _get_nc():
    global _CACHED_NC
    if _CACHED_NC is None:
        _CACHED_NC = _build_nc()
    return _CACHED_NC


def _ternarize(w):
    """Exactly the passing baseline's device semantics, on host:
    t = (w > thr) - (w < -thr), thr = 0.5*(s + EPS), s = f32(mean|w| in f64)."""
    w = np.asarray(w, dtype=np.float32)
    s = np.float32(np.mean(np.abs(w), dtype=np.float64))
    thr = np.float32(0.5) * (s + np.float32(EPS))
    t = (w > thr).astype(np.float32) - (w < -thr).astype(np.float32)
    return t, s


def run(x, w_qkv, w_proj, b_proj, trace=False):
    x = np.ascontiguousarray(x, dtype=np.float32)
    tq, s_q = _ternarize(w_qkv)
    tp, s_p = _ternarize(w_proj)
    tqT = tq.T  # [C, 3C]
    wq8T = np.ascontiguousarray(tqT[:, 0 : 2 * C]).astype(ml_dtypes.float8_e4m3fn)
    wvT = np.ascontiguousarray(tqT[:, 2 * C : 3 * C]).astype(ml_dtypes.bfloat16)
    wpT = np.ascontiguousarray(tp.T).astype(ml_dtypes.bfloat16)
    bp = np.ascontiguousarray(b_proj, dtype=np.float32)
    sq = np.array([[s_q]], dtype=np.float32)
    sp = np.array([[s_p]], dtype=np.float32)
    cz_host = np.zeros((2, N), dtype=ml_dtypes.bfloat16)
    cz_host[1, :] = 1.0

    in_maps = []
    for c in range(NCORES):
        xs = x[c * BPC : (c + 1) * BPC].reshape(T, C)
        xsT = np.ascontiguousarray(xs.T)
        in_maps.append(
            {
                "xT": xsT.astype(ml_dtypes.bfloat16),
                "x8T": xsT.astype(ml_dtypes.float8_e4m3fn),
                "wq8T": wq8T,
                "wvT": wvT,
                "wpT": wpT,
                "bp": bp,
                "sq": sq,
                "sp": sp,
                "cz": cz_host,
            }
        )

    nc = _get_nc()
    res = run_bass_kernel_spmd(
        nc, in_maps, core_ids=list(range(NCORES)), trace=trace
    )

    y = np.empty((B, N, C), dtype=np.float32)
    for c in range(NCORES):
        yT_c = res.results[c]["yT"]  # [C, T]
        y[c * BPC : (c + 1) * BPC] = yT_c.T.reshape(BPC, N, C)
    return y, res


def kernel(x, w_qkv, w_proj, b_proj):
    y, _ = run(x, w_qkv, w_proj, b_proj, trace=False)
    return y
